# revision 23
# baseline (speedup 1.0000x reference)
"""Int8 per-token-quantized linear (MluQuantLinearInt8) on 8 Trainium2 cores.

  out[s, n] = (sum_k q[s,k] * w[n,k]) * x_scale[s] * w_scale[n]
  q = round(x / x_scale) clipped to [-127, 127],  x_scale = max(|x|_row, 1e-8)/127

Sharding: data-parallel over tokens (512/core); weights replicated, streamed
once per core. The GEMM runs at the 16-bit PE roofline (~874us for 4096
matmuls/core at 216ns each), so all recoverable time is pre-GEMM startup and
the eviction tail. Startup is built around four measured constraints:
(1) transfers in flight share the ~358GB/s HBM-per-core limit, so any early
weight byte starves the quant-critical x stream, (2) HWDGE DMAs recycle 8
completion-semaphore lanes in scheduler order - a DMA can end up gated on an
unrelated slow transfer 8 slots earlier, (3) each engine is a FIFO, so a
descriptor gated on a far-future event blocks everything behind it, (4) the
PE HAM clock-gate needs ~3.4us of sustained activity to reach 2.4 GHz and
re-throttles after ~3.4us idle.

Structure:
  - x streams as 8 ungated half-tile DMAs on the Scalar queue - exactly the
    8 HWDGE lanes, so no lane-recycle stall ever blocks the Scalar FIFO and
    the first ACTIVATE runs as soon as amax is ready. The 8 qT transposes
    (Sync queue) then recycle the lanes of the early-completing x halves.
  - weight tiles all ride the gpsimd SWDGE queue (no HWDGE lanes): w0-w5
    gated behind real quant milestones (amax halves of successive tiles) via
    a tiny gpsimd copy that reads the milestone and writes into the weight
    buffer (WAW gates the DMA); the rest are paced by the 6-deep weight pool
    rotation. The x_scale broadcast DRAM round-trip also rides gpsimd so the
    Sync queue carries only transposes + output stores.
  - quantization never materializes q: the scalar engine emits
    y = fp16(x*inv + 1536) (fp16 ulp is exactly 1.0 on [1024,2048), so the
    conversion RNE-rounds to integer), y is XBAR-transposed to qT, and the
    GEMM runs fp16 x fp16 on the +1536-offset values - exact in fp32 PSUM.
    The eviction folds the offset back out with the host-precomputed
    -1536*rowsum(w) (exact in f32: 3*rs*2^9, |3*rs| < 2^24), then applies
    w_scale and x_scale: two DVE ops per output tile.
  - the GEMM starts early at half token-width: output tiles nt0-4 run tokens
    0-255 as soon as token tiles 0/1 are transposed (range-based slice deps
    let kc<16 start on the h0 halves alone); the last tile also runs as two
    halves so the final output DMA is 0.5MB.
  - an ungated burst of junk matmuls on constant tiles (~9.5us, as soon as
    the identity exists) plus milestone-gated bursts keep the PE HAM busy so
    the real GEMM starts at 2.4 GHz and never re-throttles.
"""

import sys
from contextlib import ExitStack
from functools import lru_cache

import numpy as np

for _p in ("/opt/trn_rl_repo", "/root/.axon_site/_ro/trn_rl_repo"):
    if _p not in sys.path:
        sys.path.append(_p)

import ml_dtypes  # noqa: E402

import concourse.bass as bass  # noqa: E402
import concourse.bass2jax as bass2jax  # noqa: E402
import concourse.mybir as mybir  # noqa: E402
import concourse.tile as tile  # noqa: E402
from concourse.bass_utils import (  # noqa: E402
    compile_bir_kernel as _orig_compile_bir_kernel,
    run_bass_kernel_spmd,
)
from concourse.masks import make_identity  # noqa: E402

# The walrus build in this container accepts only ONE sync-wait per
# instruction ("Too many sync wait commands", CoreV3GenImpl setupSyncWait) —
# Tile's kernel-tail drain carries several. Split extra waits onto preceding
# single-wait EventSemaphore carriers on the same engine (engine program order
# makes the AND of waits equivalent).
import json as _json  # noqa: E402


def _split_multi_waits(bir_json):
    d = _json.loads(bir_json)
    changed = False
    for fn in d.get("functions", []):
        for bb in fn.get("blocks", []) or []:
            insts = bb.get("instructions")
            if not insts:
                continue
            out = []
            for ins in insts:
                si = ins.get("sync_info")
                waits = (si or {}).get("on_wait") or []
                if len(waits) > 1:
                    for j, w in enumerate(waits[:-1]):
                        out.append(
                            {
                                "engine": ins.get("engine"),
                                "ins": [],
                                "outs": [],
                                "name": f"{ins.get('name', 'I')}_w{j}",
                                "opcode": "EventSemaphore",
                                "sync_info": {"on_update": [], "on_wait": [w]},
                            }
                        )
                    si["on_wait"] = [waits[-1]]
                    changed = True
                out.append(ins)
            bb["instructions"] = out
    if not changed:
        return bir_json
    return _json.dumps(d).encode()


def _patched_compile_bir_kernel(bir_json, tmpdir, neff_name="file.neff"):
    return _orig_compile_bir_kernel(
        _split_multi_waits(bir_json), tmpdir, neff_name=neff_name
    )


bass2jax.compile_bir_kernel = _patched_compile_bir_kernel

P = 128
NCORES = 8
S, K_FULL, N_FULL = 4096, 4096, 16384
QMAX = 127.0
# fp16 ulp is exactly 1.0 on [1024, 2048): converting x*inv + 1536 to fp16
# RNE-rounds to integer; y - 1536 recovers q (|q| <= 127.5 keeps y inside
# [1408, 1664) ⊂ [1024, 2048)).
MAGIC16 = 1536.0
F32 = mybir.dt.float32
BF16 = mybir.dt.bfloat16
FP16 = mybir.dt.float16

WBUFS = 7  # per-nt weight tiles resident in SBUF (first WBUFS prequeued)
NT_EARLY = 7  # leading output tiles run at half token-width
N_JUNK = 14  # ungated warm-up matmuls (fill ~9.5us..15us, HAM warm by ~13us)


def build_nc(S_C, K, N, warmup=True):
    """One-core program; SPMD-replicated across cores by the runner.

    Inputs (per core):
      x   [S_C, K]  f32 - this core's token slice
      wt  [NT, P, KC, P] fp16 - weights, host-packed per output-channel tile
      wsb [P, 2*NT] f32 - weight_scale & -1536*rowsum packed per channel
    Output:
      outT [N, S_C] f32 - dequantized output, transposed
    """
    KC = K // P  # contraction chunks
    KH = K // 2  # K half
    KCH = KC // 2
    TT = S_C // P  # token tiles
    SH = S_C // 2  # token half (per-core)
    NT = N // P  # output-channel tiles (one psum tile each)

    nc = bass.Bass()
    x = nc.declare_dram_parameter("x", [S_C, K], F32, isOutput=False)
    wt = nc.declare_dram_parameter("wt", [NT, P, KC, P], FP16, isOutput=False)
    wsb = nc.declare_dram_parameter("wsb", [P, 2 * NT], F32, isOutput=False)
    outT = nc.declare_dram_parameter("outT", [N, S_C], F32, isOutput=True)
    xs_scratch = nc.dram_tensor("xs_scratch", [S_C], F32)

    outT_t = outT.rearrange("(nt p) s -> nt p s", p=P)

    with tile.TileContext(nc) as tc, ExitStack() as ctx:
        const_pool = ctx.enter_context(tc.tile_pool(name="const", bufs=1))
        xpool = ctx.enter_context(tc.tile_pool(name="xp", bufs=4))
        ypool = ctx.enter_context(tc.tile_pool(name="yp", bufs=3))
        qt_pool = ctx.enter_context(tc.tile_pool(name="qt", bufs=1))
        wpool = ctx.enter_context(tc.tile_pool(name="wp", bufs=WBUFS))
        opool = ctx.enter_context(tc.tile_pool(name="op", bufs=3))
        oh_pool = ctx.enter_context(tc.tile_pool(name="ohp", bufs=6))
        spool = ctx.enter_context(tc.tile_pool(name="sp", bufs=1))
        ps_pool = ctx.enter_context(tc.tile_pool(name="psp", bufs=4, space="PSUM"))
        ph_pool = ctx.enter_context(tc.tile_pool(name="php", bufs=2, space="PSUM"))
        pt_pool = ctx.enter_context(tc.tile_pool(name="ptp", bufs=1, space="PSUM"))
        pw_pool = ctx.enter_context(tc.tile_pool(name="pwp", bufs=1, space="PSUM"))

        ident_f32 = const_pool.tile([P, P], F32)
        make_identity(nc, ident_f32)
        ident_fp16 = const_pool.tile([P, P], FP16)
        nc.vector.tensor_copy(ident_fp16, ident_f32)
        # junk rhs for the ungated warm-up burst: available as soon as the
        # vector engine can memset it (~9.5us), no DMA involved.
        junk = const_pool.tile([P, 512], FP16)
        nc.vector.memset(junk, 0.0)

        wsb_sb = const_pool.tile([P, 2 * NT], F32)
        nc.gpsimd.dma_start(wsb_sb, wsb[:, :])

        # ---- x half-tile loads: one sequential HBM stream on the Scalar
        # queue, ahead of every weight byte. 8 DMAs = the 8 HWDGE lanes, so
        # none of them (nor anything behind them on the Scalar FIFO) waits on
        # lane recycling.
        # x loads, phase 1: only t0/t1 (4 ungated half-tile DMAs on the
        # Scalar queue). Every DMA-transpose acts as a barrier against ALL
        # earlier-scheduled DMAs (XBAR serialization: a transpose waits the
        # last prior DMA on each ring, and any later DMA waits the
        # transpose), so the t0/t1 transposes must see only these four early
        # transfers ahead of them — the x tail and the weight stream are
        # released after tr t1h1 (below).
        xts = []
        for t in range(TT):
            xts.append(xpool.tile([P, K], F32, name=f"xt{t}", tag="xt"))
        for t in range(2):
            for h in range(2):
                nc.scalar.dma_start(
                    xts[t][:, h * KH : (h + 1) * KH],
                    x[t * P : (t + 1) * P, h * KH : (h + 1) * KH],
                )
        # Tiny ungated sync-queue DMA: takes the 5th HWDGE lane so the
        # transposes' lane-recycle waits all land on early transfers.
        decoy = const_pool.tile([1, P], F32)
        nc.sync.dma_start(decoy, wsb[:1, :P])

        wu_ps = (
            pw_pool.tile([P, 512], F32, name="wu_ps", tag="wu_ps") if warmup else None
        )
        if warmup:
            # Ungated junk matmuls: start as soon as junk/ident exist
            # (~9.5us), run back-to-back so the HAM SHORT window latches
            # K=8/8 by ~13us. Sized to finish before the first real MM.
            for _ in range(N_JUNK):
                nc.tensor.matmul(wu_ps, lhsT=ident_fp16, rhs=junk)

        wtiles = {}
        amaxes = [
            spool.tile([P, 1], F32, name=f"amax{t}", tag=f"amax{t}")
            for t in range(TT)
        ]

        def preload_w(nt, gate):
            # Real RAW gate: the tiny gpsimd copy reads `gate` - a slice of
            # qT written by an already-emitted transpose - and writes into
            # the weight buffer; the SWDGE DMA is WAW-ordered behind it.
            # Gating on the transpose (not an earlier milestone) matters
            # twice over: no weight byte contends with the x stream until
            # the corresponding qT data is in, and the scheduler orders the
            # weight DMA strictly after the transpose, so the transposes'
            # implicit previous-DMA waits (XBAR serialization) land on
            # early-completing transfers instead of megabyte weight tiles.
            wtile = wpool.tile([P, KC, P], FP16, name=f"wt{nt}", tag="wtile")
            nc.gpsimd.tensor_copy(wtile[:, 0, :1], gate)
            nc.gpsimd.dma_start(wtile, wt[nt])
            wtiles[nt] = wtile

        # ---- Phase 1: per-token dynamic int8 quantization + transpose ----
        # qT[k%128, t, k//128, tok%128]: each transpose target is
        # per-partition contiguous (non-contiguous dst breaks DMA transpose)
        qT = qt_pool.tile([P, TT, KC, P], FP16)
        xs_all = spool.tile([P, TT], F32)  # xs_all[p, t] = x_scale[t*128+p]
        xsb = spool.tile([P, S_C], F32, tag="xsb")
        ys = {}

        def quant_tile(t):
            xt = xts[t]
            amh = [
                spool.tile([P, 1], F32, name=f"amh{t}{h}", tag=f"amh{t}{h}")
                for h in range(2)
            ]
            for h in range(2):
                nc.vector.tensor_reduce(
                    out=amh[h],
                    in_=xt[:, h * KH : (h + 1) * KH],
                    axis=mybir.AxisListType.X,
                    op=mybir.AluOpType.max,
                    apply_absolute_value=True,
                )
            amax = amaxes[t]
            nc.vector.tensor_tensor(
                out=amax, in0=amh[0], in1=amh[1], op=mybir.AluOpType.max
            )
            # amax' = max(amax, 1e-8); x_scale = amax'/127 (~1ulp, via *1/127);
            # q = round(x * (127 * recip(amax'))) - DVE has no divide, but
            # reciprocal is bit-exact; the ~1ulp quantizer error flips a
            # rounding boundary on ~0.1 elements per 4096-row (negligible).
            nc.vector.tensor_scalar(amax, amax, 1e-8, None, op0=mybir.AluOpType.max)
            nc.vector.tensor_scalar(
                xs_all[:, t : t + 1],
                amax,
                float(np.float32(1.0 / 127.0)),
                None,
                op0=mybir.AluOpType.mult,
            )
            inv = spool.tile([P, 1], F32, name=f"inv{t}", tag=f"inv{t}")
            nc.vector.reciprocal(inv, amax)
            nc.vector.tensor_scalar(inv, inv, QMAX, None, op0=mybir.AluOpType.mult)

            y = ypool.tile([P, K], FP16, name=f"y{t}", tag="y")
            ys[t] = y
            for h in range(2):
                hs = slice(h * KH, (h + 1) * KH)
                # y = fp16(x*inv + 1536) on the scalar engine: the fp16
                # conversion RNE-rounds to integer. The GEMM runs directly on
                # the offset values; the eviction subtracts 1536*rowsum(w).
                nc.scalar.activation(
                    y[:, hs],
                    xt[:, hs],
                    mybir.ActivationFunctionType.Copy,
                    bias=MAGIC16,
                    scale=inv,
                )
                # half-tile transpose on the DMA xbar (Sync queue):
                # [tok, (kc ki)] -> [ki, kc, tok]
                nc.sync.dma_start(
                    qT[:, t, h * KCH : (h + 1) * KCH, :], y[:, hs], transpose=True
                )

        xs_ps = pt_pool.tile([2, 2 * P], F32, name="xs_ps", tag="xs_ps")

        def xs_pair(pair, gate=None):
            # x_scale for token tiles (2*pair, 2*pair+1): PE-transpose to a
            # [t, tok] layout (contiguous 512B dram rows), round-trip through
            # DRAM on the gpsimd queue (keeps Sync = transposes + outputs) to
            # broadcast [tok] across partitions. `gate` (a late qT slice)
            # keeps the scheduler from hoisting the store early in its DMA
            # order, where other DMAs would inherit waits on it.
            ts = slice(2 * pair, 2 * pair + 2)
            cols = slice(pair * P, (pair + 1) * P)
            nc.tensor.transpose(xs_ps[:, cols], xs_all[:, ts], ident_f32)
            xs_row = spool.tile([2, P], F32, name=f"xs_row{pair}", tag=f"xs_row{pair}")
            if gate is not None:
                # junk write, overwritten by the real copy below; WAW-chains
                # the whole xs round-trip behind `gate`.
                nc.gpsimd.tensor_copy(xs_row[:1, :1], gate)
            nc.vector.tensor_copy(xs_row, xs_ps[:, cols])
            nc.gpsimd.dma_start(
                xs_scratch.rearrange("(t p) -> t p", p=P)[ts, :], xs_row
            )
            nc.gpsimd.dma_start(
                xsb[:, pair * SH : (pair + 1) * SH],
                xs_scratch[None, pair * SH : (pair + 1) * SH].to_broadcast((P, SH)),
            )

        def gemm_tile(nt, span):
            # span=(first_token_tile, n_token_tiles): (0,4) full 512-token
            # width, (0,2)/(2,2) halves, (2,1)/(3,1) quarters.
            t0s, ntt = span
            if nt in wtiles:
                wtile = wtiles[nt]
            else:
                wtile = wpool.tile([P, KC, P], FP16, name=f"wt{nt}", tag="wtile")
                nc.gpsimd.dma_start(wtile, wt[nt])
                wtiles[nt] = wtile
            width = ntt * P
            rhs_t = slice(t0s, t0s + ntt)
            cols = slice(t0s * P, t0s * P + width)
            xs_in = xsb[:, cols]
            if ntt == TT:
                ps = ps_pool.tile([P, S_C], F32, name=f"ps{nt}", tag="ps")
                out_sb = opool.tile([P, S_C], F32, name=f"o{nt}", tag="out_sb")
            else:
                ps = ph_pool.tile([P, SH], F32, name=f"ph{nt}_{t0s}", tag="ph")[
                    :, :width
                ]
                out_sb = oh_pool.tile([P, SH], F32, name=f"o{nt}_{t0s}", tag="oh")
            ocols = slice(0, width)
            for kc in range(KC):
                nc.tensor.matmul(
                    ps,
                    lhsT=wtile[:, kc, :],
                    rhs=qT[:, rhs_t, kc, :],
                    start=(kc == 0),
                    stop=(kc == KC - 1),
                )
            # acc = sum_k (q+1536)*w; (acc - 1536*rowsum_n) * ws_n * xs_tok
            nc.vector.tensor_scalar(
                out_sb[:, ocols],
                ps,
                wsb_sb[:, NT + nt : NT + nt + 1],
                wsb_sb[:, nt : nt + 1],
                op0=mybir.AluOpType.add,
                op1=mybir.AluOpType.mult,
            )
            nc.vector.tensor_tensor(
                out=out_sb[:, ocols], in0=out_sb[:, ocols], in1=xs_in,
                op=mybir.AluOpType.mult,
            )
            nc.sync.dma_start(outT_t[nt][:, cols], out_sb[:, ocols])

        # Program order tuned so every engine FIFO sees monotonically-firing
        # waits: quant t0/t1, then the warm-up bridge (each burst gated on
        # the next pipeline milestone, FIFO-ordered by firing time so no
        # burst hides a later-firing one and HAM never sees a >3.4us hole),
        # then weight preloads gated on the transposes they follow, with the
        # xs0 round-trip slotted where the Pool FIFO stays monotone.
        quant_tile(0)
        quant_tile(1)
        # x loads, phase 2: t2 (one full-tile DMA), gated behind tr t1h1 via
        # a tiny scalar-engine copy (the Scalar FIFO orders the issue after
        # it). t3 is released later still (after wt4 lands) so the first
        # weight tiles get the HBM link ahead of the x tail.
        nc.scalar.activation(
            xts[2][:1, :1], qT[:1, 1, KCH, :1], mybir.ActivationFunctionType.Copy
        )
        nc.scalar.dma_start(xts[2], x[2 * P : 3 * P, :])
        if warmup:
            for t in range(2):
                for h in range(2):
                    nc.tensor.matmul(
                        wu_ps, lhsT=ident_f32, rhs=xts[t][:, h * KH : h * KH + 512]
                    )
            for reps, rhs in (
                (2, ys[0][:, 0:512]),            # y0h0 written
                (2, ys[0][:, KH : KH + 512]),    # y0h1
                (1, qT[:, 0, 0, :]),             # tr t0h0 done
                (1, qT[:, 0, KCH, :]),           # tr t0h1 done
                (2, ys[1][:, 0:512]),            # y1h0
                (2, ys[1][:, KH : KH + 512]),    # y1h1
                (1, qT[:, 1, 0, :]),             # tr t1h0 done
            ):
                for _ in range(reps):
                    nc.tensor.matmul(wu_ps[:, : rhs.free_size()],
                                     lhsT=ident_fp16, rhs=rhs)
        # All weight preloads gated behind tr t1h1: no weight byte moves (and
        # no transpose barrier forms against a weight tile) until the four
        # t0/t1 transposes are done; from there the Pool ring streams weights
        # while the x tail shares the HBM link.
        for nt in range(WBUFS):
            preload_w(nt, qT[:, 1, KCH, :1])
        xs_pair(0, gate=qT[:1, 1, KCH, :1])
        # x loads, phase 3: the t3 halves, released once wt4 is resident so
        # weight tiles w0-w4 stream at full rate first. The h1b quarter pass
        # below gives the t3 quant path ~25us of slack, so this lateness is
        # free.
        nc.scalar.activation(
            xts[3][:1, :1], wtiles[4][:1, 0, :1], mybir.ActivationFunctionType.Copy
        )
        for h in range(2):
            nc.scalar.dma_start(
                xts[3][:, h * KH : (h + 1) * KH],
                x[3 * P : 4 * P, h * KH : (h + 1) * KH],
            )
        if warmup:
            # bridge the last ~3us to the first real MM: fires on wt0 landing
            for _ in range(2):
                nc.tensor.matmul(
                    wu_ps, lhsT=wtiles[0][:, 0, :], rhs=junk
                )
        # quant of t2/t3 emitted interleaved with the leading GEMM tiles so
        # the DVE FIFO serves each PSUM eviction between reduce halves
        # (program order is FIFO order per engine; a monolithic quant block
        # would queue the first evictions behind ~10us of reduces and stall
        # the PSUM rotation).
        quant_tile(2)

        # ---- Phase 2: streamed weights-stationary GEMM + fused dequant ----
        # leading tiles run tokens 0-255 (half width) while the x tail still
        # loads, then tokens 256-383 (quarter, t2 only) while t3 quantizes,
        # then 384-511. The rest run full width; the last tile again as
        # halves so the final output DMA is 0.5MB.
        gemm_tile(0, span=(0, 2))
        gemm_tile(1, span=(0, 2))
        quant_tile(3)
        for nt in range(2, NT_EARLY):
            gemm_tile(nt, span=(0, 2))
        xs_pair(1, gate=qT[:1, 2, 0, :1])
        for nt in range(NT_EARLY):
            gemm_tile(nt, span=(2, 1))
        for nt in range(NT_EARLY):
            gemm_tile(nt, span=(3, 1))
        for nt in range(NT_EARLY, NT - 1):
            gemm_tile(nt, span=(0, TT))
        gemm_tile(NT - 1, span=(0, 2))
        gemm_tile(NT - 1, span=(2, 2))

    return nc


def pack_inputs(input_tensor, weight, weight_scale, S_C, K, N):
    """Host-side prep: shard x, pack weights to fp16 per-nt SBUF layout."""
    KC = K // P
    NT = N // P
    x = np.ascontiguousarray(input_tensor.reshape(-1, K))  # [S, K]
    w16 = weight.astype(np.float16)  # [N, K], int8 values exact
    # pack[nt, p, kc, j] = w[nt*128 + j, kc*128 + p]
    wt = np.ascontiguousarray(w16.reshape(NT, P, KC, P).transpose(0, 3, 2, 1))
    ws = weight_scale.reshape(NT, P).T.astype(np.float32)  # [P, NT]
    # -1536 * rowsum(w): 3*rs*2^9 with |3*rs| < 2^24, exact in f32
    rs = weight.astype(np.int64).sum(axis=1).astype(np.float32)
    wb = (np.float32(-1536.0) * rs).reshape(NT, P).T
    wsb = np.ascontiguousarray(np.concatenate([ws, wb], axis=1))  # [P, 2*NT]
    return x, wt, wsb


@lru_cache(maxsize=2)
def _compiled_nc(S_C, K, N, warmup):
    return build_nc(S_C, K, N, warmup=warmup)


def run(input_tensor, weight, weight_scale, n_cores=NCORES, trace=False,
        exact_divide=True, warmup=True):
    Sfull, K = input_tensor.shape[-2], input_tensor.shape[-1]
    N = weight.shape[0]
    S_C = Sfull // n_cores
    x, wt, wsb = pack_inputs(input_tensor, weight, weight_scale, S_C, K, N)
    nc = _compiled_nc(S_C, K, N, warmup)
    in_maps = [
        {"x": np.ascontiguousarray(x[c * S_C : (c + 1) * S_C]),
         "wt": wt, "wsb": wsb}
        for c in range(n_cores)
    ]
    res = run_bass_kernel_spmd(nc, in_maps, core_ids=list(range(n_cores)), trace=trace)
    out = np.empty((Sfull, N), np.float32)
    for c in range(n_cores):
        out[c * S_C : (c + 1) * S_C] = res.results[c]["outT"].T
    return out[None], res


def kernel(input_tensor, weight, weight_scale):
    out, _ = run(
        np.asarray(input_tensor), np.asarray(weight), np.asarray(weight_scale)
    )
    return out


# revision 28
# speedup vs baseline: 1.0019x; 1.0019x over previous
"""Int8 per-token-quantized linear (MluQuantLinearInt8) on 8 Trainium2 cores.

  out[s, n] = (sum_k q[s,k] * w[n,k]) * x_scale[s] * w_scale[n]
  q = round(x / x_scale) clipped to [-127, 127],  x_scale = max(|x|_row, 1e-8)/127

Sharding: data-parallel over tokens (512/core); weights replicated, streamed
once per core. The GEMM runs at the 16-bit PE roofline (~874us for 4096
matmuls/core at 216ns each), so all recoverable time is pre-GEMM startup and
the eviction tail. Startup is built around four measured constraints:
(1) transfers in flight share the ~358GB/s HBM-per-core limit, so any early
weight byte starves the quant-critical x stream, (2) HWDGE DMAs recycle 8
completion-semaphore lanes in scheduler order - a DMA can end up gated on an
unrelated slow transfer 8 slots earlier, (3) each engine is a FIFO, so a
descriptor gated on a far-future event blocks everything behind it, (4) the
PE HAM clock-gate needs ~3.4us of sustained activity to reach 2.4 GHz and
re-throttles after ~3.4us idle.

Structure:
  - x streams as 8 ungated half-tile DMAs on the Scalar queue - exactly the
    8 HWDGE lanes, so no lane-recycle stall ever blocks the Scalar FIFO and
    the first ACTIVATE runs as soon as amax is ready. The 8 qT transposes
    (Sync queue) then recycle the lanes of the early-completing x halves.
  - weight tiles all ride the gpsimd SWDGE queue (no HWDGE lanes): w0-w5
    gated behind real quant milestones (amax halves of successive tiles) via
    a tiny gpsimd copy that reads the milestone and writes into the weight
    buffer (WAW gates the DMA); the rest are paced by the 6-deep weight pool
    rotation. The x_scale broadcast DRAM round-trip also rides gpsimd so the
    Sync queue carries only transposes + output stores.
  - quantization never materializes q: the scalar engine emits
    y = fp16(x*inv + 1536) (fp16 ulp is exactly 1.0 on [1024,2048), so the
    conversion RNE-rounds to integer), y is XBAR-transposed to qT, and the
    GEMM runs fp16 x fp16 on the +1536-offset values - exact in fp32 PSUM.
    The eviction folds the offset back out with the host-precomputed
    -1536*rowsum(w) (exact in f32: 3*rs*2^9, |3*rs| < 2^24), then applies
    w_scale and x_scale: two DVE ops per output tile.
  - the GEMM starts early at half token-width: output tiles nt0-4 run tokens
    0-255 as soon as token tiles 0/1 are transposed (range-based slice deps
    let kc<16 start on the h0 halves alone); the last tile also runs as two
    halves so the final output DMA is 0.5MB.
  - an ungated burst of junk matmuls on constant tiles (~9.5us, as soon as
    the identity exists) plus milestone-gated bursts keep the PE HAM busy so
    the real GEMM starts at 2.4 GHz and never re-throttles.
"""

import sys
from contextlib import ExitStack
from functools import lru_cache

import numpy as np

for _p in ("/opt/trn_rl_repo", "/root/.axon_site/_ro/trn_rl_repo"):
    if _p not in sys.path:
        sys.path.append(_p)

import ml_dtypes  # noqa: E402

import concourse.bass as bass  # noqa: E402
import concourse.bass2jax as bass2jax  # noqa: E402
import concourse.mybir as mybir  # noqa: E402
import concourse.tile as tile  # noqa: E402
from concourse.bass_utils import (  # noqa: E402
    compile_bir_kernel as _orig_compile_bir_kernel,
    run_bass_kernel_spmd,
)
from concourse.masks import make_identity  # noqa: E402

# The walrus build in this container accepts only ONE sync-wait per
# instruction ("Too many sync wait commands", CoreV3GenImpl setupSyncWait) —
# Tile's kernel-tail drain carries several. Split extra waits onto preceding
# single-wait EventSemaphore carriers on the same engine (engine program order
# makes the AND of waits equivalent).
import json as _json  # noqa: E402


def _split_multi_waits(bir_json):
    d = _json.loads(bir_json)
    changed = False
    for fn in d.get("functions", []):
        for bb in fn.get("blocks", []) or []:
            insts = bb.get("instructions")
            if not insts:
                continue
            out = []
            for ins in insts:
                si = ins.get("sync_info")
                waits = (si or {}).get("on_wait") or []
                if len(waits) > 1:
                    for j, w in enumerate(waits[:-1]):
                        out.append(
                            {
                                "engine": ins.get("engine"),
                                "ins": [],
                                "outs": [],
                                "name": f"{ins.get('name', 'I')}_w{j}",
                                "opcode": "EventSemaphore",
                                "sync_info": {"on_update": [], "on_wait": [w]},
                            }
                        )
                    si["on_wait"] = [waits[-1]]
                    changed = True
                out.append(ins)
            bb["instructions"] = out
    if not changed:
        return bir_json
    return _json.dumps(d).encode()


def _patched_compile_bir_kernel(bir_json, tmpdir, neff_name="file.neff"):
    return _orig_compile_bir_kernel(
        _split_multi_waits(bir_json), tmpdir, neff_name=neff_name
    )


bass2jax.compile_bir_kernel = _patched_compile_bir_kernel

P = 128
NCORES = 8
S, K_FULL, N_FULL = 4096, 4096, 16384
QMAX = 127.0
# fp16 ulp is exactly 1.0 on [1024, 2048): converting x*inv + 1536 to fp16
# RNE-rounds to integer; y - 1536 recovers q (|q| <= 127.5 keeps y inside
# [1408, 1664) ⊂ [1024, 2048)).
MAGIC16 = 1536.0
F32 = mybir.dt.float32
BF16 = mybir.dt.bfloat16
FP16 = mybir.dt.float16

WBUFS = 7  # per-nt weight tiles resident in SBUF (first WBUFS prequeued)
NT_EARLY = 7  # leading output tiles run at half token-width
N_JUNK = 14  # ungated warm-up matmuls (fill ~9.5us..15us, HAM warm by ~13us)


def build_nc(S_C, K, N, warmup=True):
    """One-core program; SPMD-replicated across cores by the runner.

    Inputs (per core):
      x   [S_C, K]  f32 - this core's token slice
      wt  [NT, P, KC, P] fp16 - weights, host-packed per output-channel tile
      wsb [P, 2*NT] f32 - weight_scale & -1536*rowsum packed per channel
    Output:
      outT [N, S_C] f32 - dequantized output, transposed
    """
    KC = K // P  # contraction chunks
    KH = K // 2  # K half
    KCH = KC // 2
    TT = S_C // P  # token tiles
    SH = S_C // 2  # token half (per-core)
    NT = N // P  # output-channel tiles (one psum tile each)

    nc = bass.Bass()
    x = nc.declare_dram_parameter("x", [S_C, K], F32, isOutput=False)
    wt = nc.declare_dram_parameter("wt", [NT, P, KC, P], FP16, isOutput=False)
    wsb = nc.declare_dram_parameter("wsb", [P, 2 * NT], F32, isOutput=False)
    outT = nc.declare_dram_parameter("outT", [N, S_C], F32, isOutput=True)
    xs_scratch = nc.dram_tensor("xs_scratch", [S_C], F32)

    outT_t = outT.rearrange("(nt p) s -> nt p s", p=P)

    with tile.TileContext(nc) as tc, ExitStack() as ctx:
        const_pool = ctx.enter_context(tc.tile_pool(name="const", bufs=1))
        xpool = ctx.enter_context(tc.tile_pool(name="xp", bufs=4))
        ypool = ctx.enter_context(tc.tile_pool(name="yp", bufs=3))
        qt_pool = ctx.enter_context(tc.tile_pool(name="qt", bufs=1))
        wpool = ctx.enter_context(tc.tile_pool(name="wp", bufs=WBUFS))
        opool = ctx.enter_context(tc.tile_pool(name="op", bufs=3))
        oh_pool = ctx.enter_context(tc.tile_pool(name="ohp", bufs=6))
        spool = ctx.enter_context(tc.tile_pool(name="sp", bufs=1))
        ps_pool = ctx.enter_context(tc.tile_pool(name="psp", bufs=4, space="PSUM"))
        ph_pool = ctx.enter_context(tc.tile_pool(name="php", bufs=2, space="PSUM"))
        pt_pool = ctx.enter_context(tc.tile_pool(name="ptp", bufs=1, space="PSUM"))
        pw_pool = ctx.enter_context(tc.tile_pool(name="pwp", bufs=1, space="PSUM"))

        ident_f32 = const_pool.tile([P, P], F32)
        make_identity(nc, ident_f32)
        ident_fp16 = const_pool.tile([P, P], FP16)
        nc.vector.tensor_copy(ident_fp16, ident_f32)
        # junk rhs for the ungated warm-up burst: available as soon as the
        # vector engine can memset it (~9.5us), no DMA involved.
        junk = const_pool.tile([P, 512], FP16)
        nc.vector.memset(junk, 0.0)

        wsb_sb = const_pool.tile([P, 2 * NT], F32)
        nc.gpsimd.dma_start(wsb_sb, wsb[:, :])

        # ---- x half-tile loads: one sequential HBM stream on the Scalar
        # queue, ahead of every weight byte. 8 DMAs = the 8 HWDGE lanes, so
        # none of them (nor anything behind them on the Scalar FIFO) waits on
        # lane recycling.
        # x loads, phase 1: only t0/t1 (4 ungated half-tile DMAs on the
        # Scalar queue). Every DMA-transpose acts as a barrier against ALL
        # earlier-scheduled DMAs (XBAR serialization: a transpose waits the
        # last prior DMA on each ring, and any later DMA waits the
        # transpose), so the t0/t1 transposes must see only these four early
        # transfers ahead of them — the x tail and the weight stream are
        # released after tr t1h1 (below).
        xts = []
        for t in range(TT):
            xts.append(xpool.tile([P, K], F32, name=f"xt{t}", tag="xt"))
        for t in range(2):
            for h in range(2):
                nc.scalar.dma_start(
                    xts[t][:, h * KH : (h + 1) * KH],
                    x[t * P : (t + 1) * P, h * KH : (h + 1) * KH],
                )
        # Tiny ungated sync-queue DMA: takes the 5th HWDGE lane so the
        # transposes' lane-recycle waits all land on early transfers.
        decoy = const_pool.tile([1, P], F32)
        nc.sync.dma_start(decoy, wsb[:1, :P])

        wu_ps = (
            pw_pool.tile([P, 512], F32, name="wu_ps", tag="wu_ps") if warmup else None
        )
        if warmup:
            # Ungated junk matmuls: start as soon as junk/ident exist
            # (~9.5us), run back-to-back so the HAM SHORT window latches
            # K=8/8 by ~13us. Sized to finish before the first real MM.
            for _ in range(N_JUNK):
                nc.tensor.matmul(wu_ps, lhsT=ident_fp16, rhs=junk)

        wtiles = {}
        amaxes = [
            spool.tile([P, 1], F32, name=f"amax{t}", tag=f"amax{t}")
            for t in range(TT)
        ]

        def preload_w(nt, gate):
            # Real RAW gate: the tiny gpsimd copy reads `gate` - a slice of
            # qT written by an already-emitted transpose - and writes into
            # the weight buffer; the SWDGE DMA is WAW-ordered behind it.
            # Gating on the transpose (not an earlier milestone) matters
            # twice over: no weight byte contends with the x stream until
            # the corresponding qT data is in, and the scheduler orders the
            # weight DMA strictly after the transpose, so the transposes'
            # implicit previous-DMA waits (XBAR serialization) land on
            # early-completing transfers instead of megabyte weight tiles.
            wtile = wpool.tile([P, KC, P], FP16, name=f"wt{nt}", tag="wtile")
            nc.gpsimd.tensor_copy(wtile[:, 0, :1], gate)
            # Sync HWDGE ring: the weight bytes drain in ring-FIFO order
            # right behind the t0/t1 transposes at full link rate - no SWDGE
            # first-byte latency, no fair-share against the x tail.
            nc.sync.dma_start(wtile, wt[nt])
            wtiles[nt] = wtile

        # ---- Phase 1: per-token dynamic int8 quantization + transpose ----
        # qT[k%128, t, k//128, tok%128]: each transpose target is
        # per-partition contiguous (non-contiguous dst breaks DMA transpose)
        qT = qt_pool.tile([P, TT, KC, P], FP16)
        xs_all = spool.tile([P, TT], F32)  # xs_all[p, t] = x_scale[t*128+p]
        xsb = spool.tile([P, S_C], F32, tag="xsb")
        ys = {}

        def quant_tile(t):
            xt = xts[t]
            amh = [
                spool.tile([P, 1], F32, name=f"amh{t}{h}", tag=f"amh{t}{h}")
                for h in range(2)
            ]
            for h in range(2):
                nc.vector.tensor_reduce(
                    out=amh[h],
                    in_=xt[:, h * KH : (h + 1) * KH],
                    axis=mybir.AxisListType.X,
                    op=mybir.AluOpType.max,
                    apply_absolute_value=True,
                )
            amax = amaxes[t]
            nc.vector.tensor_tensor(
                out=amax, in0=amh[0], in1=amh[1], op=mybir.AluOpType.max
            )
            # amax' = max(amax, 1e-8); x_scale = amax'/127 (~1ulp, via *1/127);
            # q = round(x * (127 * recip(amax'))) - DVE has no divide, but
            # reciprocal is bit-exact; the ~1ulp quantizer error flips a
            # rounding boundary on ~0.1 elements per 4096-row (negligible).
            nc.vector.tensor_scalar(amax, amax, 1e-8, None, op0=mybir.AluOpType.max)
            nc.vector.tensor_scalar(
                xs_all[:, t : t + 1],
                amax,
                float(np.float32(1.0 / 127.0)),
                None,
                op0=mybir.AluOpType.mult,
            )
            inv = spool.tile([P, 1], F32, name=f"inv{t}", tag=f"inv{t}")
            nc.vector.reciprocal(inv, amax)
            nc.vector.tensor_scalar(inv, inv, QMAX, None, op0=mybir.AluOpType.mult)

            y = ypool.tile([P, K], FP16, name=f"y{t}", tag="y")
            ys[t] = y
            for h in range(2):
                hs = slice(h * KH, (h + 1) * KH)
                # y = fp16(x*inv + 1536) on the scalar engine: the fp16
                # conversion RNE-rounds to integer. The GEMM runs directly on
                # the offset values; the eviction subtracts 1536*rowsum(w).
                nc.scalar.activation(
                    y[:, hs],
                    xt[:, hs],
                    mybir.ActivationFunctionType.Copy,
                    bias=MAGIC16,
                    scale=inv,
                )
                # half-tile transpose on the DMA xbar (Sync queue):
                # [tok, (kc ki)] -> [ki, kc, tok]
                nc.sync.dma_start(
                    qT[:, t, h * KCH : (h + 1) * KCH, :], y[:, hs], transpose=True
                )

        xs_ps = pt_pool.tile([2, 2 * P], F32, name="xs_ps", tag="xs_ps")

        def xs_pair(pair, gate=None):
            # x_scale for token tiles (2*pair, 2*pair+1): PE-transpose to a
            # [t, tok] layout (contiguous 512B dram rows), round-trip through
            # DRAM on the gpsimd queue (keeps Sync = transposes + outputs) to
            # broadcast [tok] across partitions. `gate` (a late qT slice)
            # keeps the scheduler from hoisting the store early in its DMA
            # order, where other DMAs would inherit waits on it.
            ts = slice(2 * pair, 2 * pair + 2)
            cols = slice(pair * P, (pair + 1) * P)
            nc.tensor.transpose(xs_ps[:, cols], xs_all[:, ts], ident_f32)
            xs_row = spool.tile([2, P], F32, name=f"xs_row{pair}", tag=f"xs_row{pair}")
            if gate is not None:
                # junk write, overwritten by the real copy below; WAW-chains
                # the whole xs round-trip behind `gate`.
                nc.gpsimd.tensor_copy(xs_row[:1, :1], gate)
            nc.vector.tensor_copy(xs_row, xs_ps[:, cols])
            nc.gpsimd.dma_start(
                xs_scratch.rearrange("(t p) -> t p", p=P)[ts, :], xs_row
            )
            nc.gpsimd.dma_start(
                xsb[:, pair * SH : (pair + 1) * SH],
                xs_scratch[None, pair * SH : (pair + 1) * SH].to_broadcast((P, SH)),
            )

        def gemm_tile(nt, span):
            # span=(first_token_tile, n_token_tiles): (0,4) full 512-token
            # width, (0,2)/(2,2) halves, (2,1)/(3,1) quarters.
            t0s, ntt = span
            if nt in wtiles:
                wtile = wtiles[nt]
            else:
                wtile = wpool.tile([P, KC, P], FP16, name=f"wt{nt}", tag="wtile")
                nc.gpsimd.dma_start(wtile, wt[nt])
                wtiles[nt] = wtile
            width = ntt * P
            rhs_t = slice(t0s, t0s + ntt)
            cols = slice(t0s * P, t0s * P + width)
            xs_in = xsb[:, cols]
            if ntt == TT:
                ps = ps_pool.tile([P, S_C], F32, name=f"ps{nt}", tag="ps")
                out_sb = opool.tile([P, S_C], F32, name=f"o{nt}", tag="out_sb")
            else:
                ps = ph_pool.tile([P, SH], F32, name=f"ph{nt}_{t0s}", tag="ph")[
                    :, :width
                ]
                out_sb = oh_pool.tile([P, SH], F32, name=f"o{nt}_{t0s}", tag="oh")
            ocols = slice(0, width)
            for kc in range(KC):
                nc.tensor.matmul(
                    ps,
                    lhsT=wtile[:, kc, :],
                    rhs=qT[:, rhs_t, kc, :],
                    start=(kc == 0),
                    stop=(kc == KC - 1),
                )
            # acc = sum_k (q+1536)*w; (acc - 1536*rowsum_n) * ws_n * xs_tok
            nc.vector.tensor_scalar(
                out_sb[:, ocols],
                ps,
                wsb_sb[:, NT + nt : NT + nt + 1],
                wsb_sb[:, nt : nt + 1],
                op0=mybir.AluOpType.add,
                op1=mybir.AluOpType.mult,
            )
            nc.vector.tensor_tensor(
                out=out_sb[:, ocols], in0=out_sb[:, ocols], in1=xs_in,
                op=mybir.AluOpType.mult,
            )
            # outputs ride the gpsimd SWDGE ring, keeping the Sync ring =
            # transposes + the seven preloaded weight tiles only.
            nc.gpsimd.dma_start(outT_t[nt][:, cols], out_sb[:, ocols])

        # Program order tuned so every engine FIFO sees monotonically-firing
        # waits: quant t0/t1, then the warm-up bridge (each burst gated on
        # the next pipeline milestone, FIFO-ordered by firing time so no
        # burst hides a later-firing one and HAM never sees a >3.4us hole),
        # then weight preloads gated on the transposes they follow, with the
        # xs0 round-trip slotted where the Pool FIFO stays monotone.
        quant_tile(0)
        quant_tile(1)
        # x loads, phase 2: t2 (one full-tile DMA), gated behind tr t1h1 via
        # a tiny scalar-engine copy (the Scalar FIFO orders the issue after
        # it). t3 is released later still (after wt4 lands) so the first
        # weight tiles get the HBM link ahead of the x tail.
        nc.scalar.activation(
            xts[2][:1, :1], qT[:1, 1, KCH, :1], mybir.ActivationFunctionType.Copy
        )
        nc.scalar.dma_start(xts[2], x[2 * P : 3 * P, :])
        if warmup:
            for t in range(2):
                for h in range(2):
                    nc.tensor.matmul(
                        wu_ps, lhsT=ident_f32, rhs=xts[t][:, h * KH : h * KH + 512]
                    )
            for reps, rhs in (
                (2, ys[0][:, 0:512]),            # y0h0 written
                (2, ys[0][:, KH : KH + 512]),    # y0h1
                (1, qT[:, 0, 0, :]),             # tr t0h0 done
                (1, qT[:, 0, KCH, :]),           # tr t0h1 done
                (2, ys[1][:, 0:512]),            # y1h0
                (2, ys[1][:, KH : KH + 512]),    # y1h1
                (1, qT[:, 1, 0, :]),             # tr t1h0 done
                (1, qT[:, 1, KCH, :]),           # tr t1h1 done
            ):
                for _ in range(reps):
                    nc.tensor.matmul(wu_ps[:, : rhs.free_size()],
                                     lhsT=ident_fp16, rhs=rhs)
        # All weight preloads gated behind tr t1h1: no weight byte moves (and
        # no transpose barrier forms against a weight tile) until the four
        # t0/t1 transposes are done; from there the Pool ring streams weights
        # while the x tail shares the HBM link.
        for nt in range(WBUFS):
            preload_w(nt, qT[:, 1, KCH, :1])
        xs_pair(0, gate=qT[:1, 1, KCH, :1])
        # x loads, phase 3: the t3 halves, released once wt4 is resident so
        # weight tiles w0-w4 stream at full rate first. The h1b quarter pass
        # below gives the t3 quant path ~25us of slack, so this lateness is
        # free.
        nc.scalar.activation(
            # kc=1 slice: written ONLY by the wt4 DMA (the gate-copy touches
            # kc=0), so this binds to the weight DATA landing, not the copy.
            xts[3][:1, :1], wtiles[4][:1, 1, :1],
            mybir.ActivationFunctionType.Copy,
        )
        for h in range(2):
            nc.scalar.dma_start(
                xts[3][:, h * KH : (h + 1) * KH],
                x[3 * P : 4 * P, h * KH : (h + 1) * KH],
            )
        if warmup:
            # bridge the last ~3us to the first real MM: fires on wt0 landing
            for _ in range(2):
                nc.tensor.matmul(
                    wu_ps, lhsT=wtiles[0][:, 0, :], rhs=junk
                )
        # quant of t2/t3 emitted interleaved with the leading GEMM tiles so
        # the DVE FIFO serves each PSUM eviction between reduce halves
        # (program order is FIFO order per engine; a monolithic quant block
        # would queue the first evictions behind ~10us of reduces and stall
        # the PSUM rotation).
        quant_tile(2)

        # ---- Phase 2: streamed weights-stationary GEMM + fused dequant ----
        # leading tiles run tokens 0-255 (half width) while the x tail still
        # loads, then tokens 256-383 (quarter, t2 only) while t3 quantizes,
        # then 384-511. The rest run full width; the last tile again as
        # halves so the final output DMA is 0.5MB.
        gemm_tile(0, span=(0, 2))
        gemm_tile(1, span=(0, 2))
        quant_tile(3)
        for nt in range(2, NT_EARLY):
            gemm_tile(nt, span=(0, 2))
        xs_pair(1, gate=qT[:1, 2, 0, :1])
        for nt in range(NT_EARLY):
            gemm_tile(nt, span=(2, 1))
        for nt in range(NT_EARLY):
            gemm_tile(nt, span=(3, 1))
        for nt in range(NT_EARLY, NT - 1):
            gemm_tile(nt, span=(0, TT))
        gemm_tile(NT - 1, span=(0, 2))
        gemm_tile(NT - 1, span=(2, 2))

    return nc


def pack_inputs(input_tensor, weight, weight_scale, S_C, K, N):
    """Host-side prep: shard x, pack weights to fp16 per-nt SBUF layout."""
    KC = K // P
    NT = N // P
    x = np.ascontiguousarray(input_tensor.reshape(-1, K))  # [S, K]
    w16 = weight.astype(np.float16)  # [N, K], int8 values exact
    # pack[nt, p, kc, j] = w[nt*128 + j, kc*128 + p]
    wt = np.ascontiguousarray(w16.reshape(NT, P, KC, P).transpose(0, 3, 2, 1))
    ws = weight_scale.reshape(NT, P).T.astype(np.float32)  # [P, NT]
    # -1536 * rowsum(w): 3*rs*2^9 with |3*rs| < 2^24, exact in f32
    rs = weight.astype(np.int64).sum(axis=1).astype(np.float32)
    wb = (np.float32(-1536.0) * rs).reshape(NT, P).T
    wsb = np.ascontiguousarray(np.concatenate([ws, wb], axis=1))  # [P, 2*NT]
    return x, wt, wsb


@lru_cache(maxsize=2)
def _compiled_nc(S_C, K, N, warmup):
    return build_nc(S_C, K, N, warmup=warmup)


def run(input_tensor, weight, weight_scale, n_cores=NCORES, trace=False,
        exact_divide=True, warmup=True):
    Sfull, K = input_tensor.shape[-2], input_tensor.shape[-1]
    N = weight.shape[0]
    S_C = Sfull // n_cores
    x, wt, wsb = pack_inputs(input_tensor, weight, weight_scale, S_C, K, N)
    nc = _compiled_nc(S_C, K, N, warmup)
    in_maps = [
        {"x": np.ascontiguousarray(x[c * S_C : (c + 1) * S_C]),
         "wt": wt, "wsb": wsb}
        for c in range(n_cores)
    ]
    res = run_bass_kernel_spmd(nc, in_maps, core_ids=list(range(n_cores)), trace=trace)
    out = np.empty((Sfull, N), np.float32)
    for c in range(n_cores):
        out[c * S_C : (c + 1) * S_C] = res.results[c]["outT"].T
    return out[None], res


def kernel(input_tensor, weight, weight_scale):
    out, _ = run(
        np.asarray(input_tensor), np.asarray(weight), np.asarray(weight_scale)
    )
    return out
